# revision 1
# baseline (speedup 1.0000x reference)
"""Trainium2 Bass kernel for nn_BinaryQuantumClassifier.

Math: the 4-qubit circuit collapses to a closed form. Per sample, with
theta_j = pi * (x @ W_ctq.T + b_ctq)_j  (j = 4r + i, reuse r, qubit i):
    d_i(theta) = a_i + b_i sin(theta) + c_i cos(theta)
              = a_i + R_i sin(pi * (y + b_ctq_j + phi_i/pi))
(R = hypot(b, c), phi = atan2(c, b); a/b/c derived from the fixed per-qubit
unitary RZ RY RX after RY(theta) H|0>), and the CNOT chain maps
Z-expectations to products of the d_i:
    z0 = d1 d2 d3, z1 = d0 d1, z2 = d0 d1 d2, z3 = d0 d1 d2 d3.
Output = (mean over r of z) @ W_cls.T + b_cls.

Device plan per core (8192 samples). x is relayouted on the host so the PE
uses it as the STATIONARY operand, split into bf16 hi+lo (x = xhi + xlo,
W = Whi + Wlo; 3 passes hi*hi + hi*lo + lo*hi accumulated in fp32 PSUM —
bf16 weight loads get FWL, 4x faster than fp32):
  lhsT = x-chunk [128 D x 128 samples], rhs = W-chunk [128 D x 8],
  out[128 samples, 8]; 8 sample-groups share one PSUM bank tile [128, 64].
A DVE tensor_add per bank assembles E (+ the phase-shift constant) with
free = j*32 + u, in sample-half tiles E_h [128, 256] (n = 128*(32h + u) + p).
Epilogue per half (overlaps the other half's matmuls):
  k2 = ((ysh + 1.5*2^24) - 1.5*2^24) rounds to the nearest even integer
  (exact range reduction), rsh = ysh - k2 in [-1, 1], ScalarE Sin once,
  d = a + R sin, CNOT products, mean over r, final 4->2 linear; one
  [128, 128] output tile.
"""

import numpy as np

import concourse.bass as bass
import concourse.mybir as mybir
from concourse import bass_utils
from concourse.tile import TileContext

B, D, NQ = 65536, 512, 4
NCORES = 8
BC = B // NCORES            # 8192 samples per core
NCH = D // 128              # 4 K-chunks
NS = 32                     # slabs per core
F = BC // NS                # 256 samples per slab
FE = BC // 128              # 64 samples per partition in epilogue layout
M2 = float(np.float32(1.5 * 2 ** 24))   # round-to-even-integer magic
PI = float(np.pi)
MM_DT = mybir.dt.bfloat16   # PE operand dtype: x/W split into bf16 hi+lo
                            # (3 passes hi*hi + hi*lo + lo*hi, fp32 PSUM accum)
NG = BC // 128              # 64 sample-groups per core
GPB = 8                     # groups per PSUM bank tile
AL = mybir.AluOpType
AF = mybir.ActivationFunctionType
F32 = mybir.dt.float32


def _split_waits(nc, max_waits=1):
    """walrus in this env accepts at most one sync-wait per instruction;
    move extras onto preceding same-engine NoOps."""
    for fn in nc.m.functions:
        for blk in fn.blocks:
            new_list = []
            for inst in blk.instructions:
                si = inst.sync_info
                if si is not None and len(si.on_wait) > max_waits:
                    waits = list(si.on_wait)
                    keep, extra = waits[-max_waits:], waits[:-max_waits]
                    for k, w in enumerate(extra):
                        new_list.append(mybir.InstNoOp(
                            name=f"{inst.name}-ws{k}", engine=inst.engine,
                            ins=[], outs=[],
                            sync_info=mybir.SyncInfo(on_wait=[w], on_update=[])))
                    si.on_wait = keep
                    inst.sync_info = si
                new_list.append(inst)
            blk.instructions = new_list


def _build_nc():
    nc = bass.Bass("TRN2", target_bir_lowering=False)
    # x relayout, hi/lo interleaved per 8-group block:
    # xa[p, g*(2*GL) + {0:GL}=hi | {GL:2GL}=lo], inner = m*512 + k*128 + ms
    xa_d = nc.dram_tensor("xa", [128, 2 * BC * NCH], MM_DT, kind="ExternalInput").ap()
    whi_d = nc.dram_tensor("whi", [D, 8], MM_DT, kind="ExternalInput").ap()
    wlo_d = nc.dram_tensor("wlo", [D, 8], MM_DT, kind="ExternalInput").ap()
    cv_d = nc.dram_tensor("cv", [128, 16], F32, kind="ExternalInput").ap()
    # cvt: [bsT | RT | aT], each [128, 256] laid out j*32 + u
    FH = FE // 2              # 32 sample-groups per half
    JH = 8 * FH               # 256: width of a half tile
    cvt_d = nc.dram_tensor("cvt", [128, 3 * JH], F32, kind="ExternalInput").ap()
    o_d = nc.dram_tensor("o", [128, 2 * FE], F32, kind="ExternalOutput").ap()

    with TileContext(nc) as tc:
        with tc.tile_pool(name="wp", bufs=1) as wpool, \
             tc.tile_pool(name="xp", bufs=8) as xpool, \
             tc.tile_pool(name="pp", bufs=4, space="PSUM") as pspool, \
             tc.tile_pool(name="ep", bufs=1) as epool:
            whis, wlos = [], []
            for k in range(NCH):
                whik = wpool.tile([128, 8], MM_DT, name=f"whik{k}")
                nc.sync.dma_start(whik[:], whi_d[128 * k:128 * (k + 1), :])
                whis.append(whik)
                wlok = wpool.tile([128, 8], MM_DT, name=f"wlok{k}")
                nc.sync.dma_start(wlok[:], wlo_d[128 * k:128 * (k + 1), :])
                wlos.append(wlok)
            cv = wpool.tile([128, 16], F32)
            nc.sync.dma_start(cv[:], cv_d[:])
            cvt = wpool.tile([128, 3 * JH], F32)
            nc.sync.dma_start(cvt[:], cvt_d[:])
            bsT = cvt[:, 0:JH]
            RT, aT = cvt[:, JH:2 * JH], cvt[:, 2 * JH:3 * JH]
            bs3 = bsT.rearrange("p (j u) -> p j u", j=8)

            O2 = epool.tile([128, 2 * FE], F32)
            GL = GPB * NCH * 128          # 4096: L-tile free width (8 groups)
            for h in range(2):
                # E_h: partition p = sample-in-group, free = j*FH + u
                E = epool.tile([128, JH], F32, name=f"E{h}")
                e3 = E.rearrange("p (j u) -> p j u", j=8)
                for gg in range(4):
                    g = 4 * h + gg
                    La = xpool.tile([128, 2 * GL], MM_DT, tag="La", name=f"La{g}")
                    nc.sync.dma_start(La[:], xa_d[:, g * 2 * GL:(g + 1) * 2 * GL])
                    Lhi = La[:, 0:GL]
                    Llo = La[:, GL:2 * GL]
                    ps = pspool.tile([128, GPB * 8], F32, tag="ps", name=f"ps{g}")
                    for mm in range(GPB):
                        for k in range(NCH):
                            off = mm * (NCH * 128) + k * 128
                            out_sl = ps[:, 8 * mm:8 * mm + 8]
                            nc.tensor.matmul(out_sl, Lhi[:, off:off + 128], whis[k][:],
                                             start=(k == 0), stop=False)
                            nc.tensor.matmul(out_sl, Lhi[:, off:off + 128], wlos[k][:],
                                             start=False, stop=False)
                            nc.tensor.matmul(out_sl, Llo[:, off:off + 128], whis[k][:],
                                             start=False, stop=(k == NCH - 1))
                    # E[:, j*FH + 8gg + m] = ps[:, 8m + j] + bs (phase shift)
                    nc.vector.tensor_add(e3[:, :, GPB * gg:GPB * (gg + 1)],
                                         ps.rearrange("p (m j) -> p j m", j=8),
                                         bs3[:, :, GPB * gg:GPB * (gg + 1)])

                # ---- epilogue for this half ----
                k2 = epool.tile([128, JH], F32, name=f"k2_{h}")
                r_ = epool.tile([128, JH], F32, name=f"r_{h}")
                s_ = epool.tile([128, JH], F32, name=f"s_{h}")
                t1 = epool.tile([128, JH], F32, name=f"t1_{h}")
                d_ = epool.tile([128, JH], F32, name=f"d_{h}")
                nc.vector.tensor_scalar(k2[:], E[:], M2, M2, AL.add, AL.subtract)
                nc.vector.tensor_sub(r_[:], E[:], k2[:])       # ysh mod 2 -> [-1, 1]
                nc.scalar.activation(s_[:], r_[:], AF.Sin, scale=PI)
                nc.vector.tensor_mul(t1[:], s_[:], RT)         # R sin
                nc.vector.tensor_add(d_[:], t1[:], aT)         # d = a + R sin

                def dj(r, i):
                    j = 4 * r + i
                    return d_[:, j * FH:(j + 1) * FH]

                # products: z_k^r at Z[:, (2k + r)*FH]
                S_ = epool.tile([128, 2 * FH], F32, name=f"S_{h}")
                Z_ = epool.tile([128, 8 * FH], F32, name=f"Z_{h}")

                def zs(k, r):
                    return Z_[:, (2 * k + r) * FH:(2 * k + r + 1) * FH]

                for r in range(2):
                    u_ = S_[:, r * FH:(r + 1) * FH]
                    nc.vector.tensor_mul(u_, dj(r, 1), dj(r, 2))        # d1 d2
                    nc.vector.tensor_mul(zs(1, r), dj(r, 0), dj(r, 1))  # z1
                    nc.vector.tensor_mul(zs(2, r), dj(r, 0), u_)        # z2
                    nc.vector.tensor_mul(zs(0, r), u_, dj(r, 3))        # z0
                    nc.vector.tensor_mul(zs(3, r), zs(2, r), dj(r, 3))  # z3
                Mn = epool.tile([128, 4 * FH], F32, name=f"Mn{h}")
                for k in range(4):
                    nc.vector.tensor_add(Mn[:, k * FH:(k + 1) * FH], zs(k, 0), zs(k, 1))

                # final linear: W' = 0.5*W_cls via cv columns
                O1 = epool.tile([128, 2 * FH], F32, name=f"O1_{h}")

                def mk(k):
                    return Mn[:, k * FH:(k + 1) * FH]

                for c in range(2):
                    o1a = O1[:, c * FH:(c + 1) * FH]
                    o1b = S_[:, c * FH:(c + 1) * FH]    # reuse S_ as scratch
                    o2 = O2[:, c * FE + FH * h:c * FE + FH * (h + 1)]
                    nc.vector.tensor_scalar(o1a, mk(0), cv[:, 4 + 4 * c:5 + 4 * c],
                                            cv[:, 12 + c:13 + c], AL.mult, AL.add)
                    nc.vector.scalar_tensor_tensor(o1b, mk(1), cv[:, 5 + 4 * c:6 + 4 * c],
                                                   o1a, AL.mult, AL.add)
                    nc.vector.scalar_tensor_tensor(o1a, mk(2), cv[:, 6 + 4 * c:7 + 4 * c],
                                                   o1b, AL.mult, AL.add)
                    nc.vector.scalar_tensor_tensor(o2, mk(3), cv[:, 7 + 4 * c:8 + 4 * c],
                                                   o1a, AL.mult, AL.add)
            nc.sync.dma_start(o_d[:], O2[:])

    return nc


_NC_CACHE = {}


def _get_nc(split=True):
    key = ("nc", split)
    if key not in _NC_CACHE:
        nc = _build_nc()
        if split:
            _split_waits(nc)
        _NC_CACHE[key] = nc
    return _NC_CACHE[key]


def _qubit_abc(q_params):
    """Exact (a_i, b_i, c_i) with d_i(theta) = a + b sin(theta) + c cos(theta)."""
    out = np.zeros((NQ, 3), np.float64)
    for i in range(NQ):
        pa, pb, pc = [float(v) for v in q_params[3 * i:3 * i + 3]]

        def rx(t):
            return np.array([[np.cos(t / 2), -1j * np.sin(t / 2)],
                             [-1j * np.sin(t / 2), np.cos(t / 2)]])

        def ry(t):
            return np.array([[np.cos(t / 2), -np.sin(t / 2)],
                             [np.sin(t / 2), np.cos(t / 2)]])

        def rz(t):
            return np.array([[np.exp(-0.5j * t), 0], [0, np.exp(0.5j * t)]])

        H = np.array([[1, 1], [1, -1]]) / np.sqrt(2)
        U = rz(pc) @ ry(pb) @ rx(pa)

        def dfun(theta):
            v = U @ ry(theta) @ H @ np.array([1.0, 0.0])
            pr = np.abs(v) ** 2
            return pr[0] - pr[1]

        d0, dpi, dh = dfun(0.0), dfun(np.pi), dfun(np.pi / 2)
        a = (d0 + dpi) / 2
        c = (d0 - dpi) / 2
        b = dh - a
        out[i] = (a, b, c)
    return out


def _make_consts(b_ctq, q_params, W_cls, b_cls):
    abc = _qubit_abc(q_params)
    cv = np.zeros((128, 16), np.float32)
    wp = 0.5 * np.asarray(W_cls, np.float64)      # mean over r folded in
    for c in range(2):
        for k in range(4):
            cv[:, 4 + 4 * c + k] = np.float32(wp[c, k])
        cv[:, 12 + c] = np.float32(b_cls[c])
    FH = FE // 2
    JH = 8 * FH
    cvt = np.zeros((128, 3 * JH), np.float32)
    for j in range(8):
        i = j % 4
        a, b, c_ = abc[i]
        R = np.hypot(b, c_)
        phi = np.arctan2(c_, b)
        cvt[:, 0 * JH + j * FH:0 * JH + (j + 1) * FH] = np.float32(b_ctq[j] + phi / np.pi)
        cvt[:, 1 * JH + j * FH:1 * JH + (j + 1) * FH] = np.float32(R)
        cvt[:, 2 * JH + j * FH:2 * JH + (j + 1) * FH] = np.float32(a)
    return cv, cvt


def make_in_maps(x, W_ctq, b_ctq, q_params, W_cls, b_cls):
    import ml_dtypes
    bf16 = ml_dtypes.bfloat16
    wt = np.asarray(W_ctq, np.float32).T                        # [512, 8]
    whi = wt.astype(bf16)
    wlo = (wt - whi.astype(np.float32)).astype(bf16)
    cv, cvt = _make_consts(np.asarray(b_ctq, np.float32),
                           np.asarray(q_params, np.float32),
                           np.asarray(W_cls, np.float32),
                           np.asarray(b_cls, np.float32))
    x = np.asarray(x, np.float32)
    in_maps = []
    for c in range(NCORES):
        xs = x[c * BC:(c + 1) * BC]                             # [8192, 512]
        # relayout: [p, m*512 + k*128 + ms] = xs[128 m + ms, 128 k + p]
        xt = xs.reshape(NG, 128, NCH, 128).transpose(3, 0, 2, 1).reshape(128, BC * NCH)
        xhi = xt.astype(bf16)
        xlo = (xt - xhi.astype(np.float32)).astype(bf16)
        GL = GPB * NCH * 128
        xa = np.concatenate(
            [np.stack([xhi.reshape(128, NG // GPB, GL)[:, g],
                       xlo.reshape(128, NG // GPB, GL)[:, g]], axis=1)
             for g in range(NG // GPB)], axis=1).reshape(128, 2 * BC * NCH)
        xa = np.ascontiguousarray(xa)
        in_maps.append({"xa": xa, "whi": whi, "wlo": wlo, "cv": cv, "cvt": cvt})
    return in_maps


def assemble_output(results):
    out = np.empty((B, 2), np.float32)
    for core in range(NCORES):
        o = results[core]["o"]                                   # [128, 2*FE]
        for c in range(2):
            # o[p, c*FE + u] = out_c(sample 128 u + p)
            out[core * BC:(core + 1) * BC, c] = \
                o[:, c * FE:(c + 1) * FE].T.reshape(BC)
    return out


def kernel(x, W_ctq, b_ctq, q_params, W_cls, b_cls):
    nc = _get_nc()
    in_maps = make_in_maps(x, W_ctq, b_ctq, q_params, W_cls, b_cls)
    res = bass_utils.run_bass_kernel_spmd(nc, in_maps, core_ids=list(range(NCORES)))
    return assemble_output(res.results)



# revision 2
# speedup vs baseline: 1.8355x; 1.8355x over previous
"""Trainium2 Bass kernel for nn_BinaryQuantumClassifier.

Math: the 4-qubit circuit collapses to a closed form. Per sample, with
theta_j = pi * (x @ W_ctq.T + b_ctq)_j  (j = 4r + i, reuse r, qubit i):
    d_i(theta) = a_i + b_i sin(theta) + c_i cos(theta)
              = a_i + R_i sin(pi * (y + b_ctq_j + phi_i/pi))
(R = hypot(b, c), phi = atan2(c, b); a/b/c derived from the fixed per-qubit
unitary RZ RY RX after RY(theta) H|0>), and the CNOT chain maps
Z-expectations to products of the d_i:
    z0 = d1 d2 d3, z1 = d0 d1, z2 = d0 d1 d2, z3 = d0 d1 d2 d3.
Output = (mean over r of z) @ W_cls.T + b_cls.

Device plan per core (8192 samples). The kernel is HBM-bound on reading x,
so x is sent as fp16 (2 B/elem, ~8.4 MB/core; fp16's 10 mantissa bits keep
the final rel err ~2e-3, well under the gate). x is relayouted on the host
so the PE uses it as the STATIONARY operand (FWL: fast weight load for
16-bit), W as the tiny moving operand:
  lhsT = x-chunk [128 D x 128 samples], rhs = W-chunk [128 D x 8] fp16,
  out[128 samples, 8] accumulated over 4 D-chunks in fp32 PSUM.
8 sample-groups share one PSUM tile [128, 64]; one 1-MB DMA per 8-group
block, all issued up-front on the sync ring (sequential queue => data
streams continuously at the HBM roofline while the PE/DVE work under its
shadow). Constants are packed into 2 DMAs on the ACT ring.
A DVE tensor_add per block assembles E (+ the phase-shift constant) with
free = j*16 + u, in per-quarter tiles E_q [128, 128] (n = 128*(16q + u) + p).
Epilogue per quarter of 16 groups (overlaps later blocks' DMA/matmuls):
  k2 = ((E + 1.5*2^24) - 1.5*2^24) rounds to the nearest even integer
  (exact range reduction), r = E - k2 in [-1, 1], ScalarE Sin once,
  d = a + R sin, CNOT products, mean over r, final 4->2 linear; one
  [128, 32] output tile DMA'd out per quarter on the ACT ring.
"""

import numpy as np

import concourse.bass as bass
import concourse.mybir as mybir
from concourse import bass_utils
from concourse.tile import TileContext

B, D, NQ = 65536, 512, 4
NCORES = 8
BC = B // NCORES            # 8192 samples per core
NCH = D // 128              # 4 K-chunks
FE = BC // 128              # 64 sample-groups per core (epilogue u index)
M2 = float(np.float32(1.5 * 2 ** 24))   # round-to-even-integer magic
PI = float(np.pi)
MM_DT = mybir.dt.float16    # PE operand dtype (x and W both fp16)
NG = BC // 128              # 64 sample-groups per core
GPB = 8                     # groups per block (per x DMA / PSUM tile)
NBLK = NG // GPB            # 8 blocks
NQT = 4                     # epilogue quarters
FQ = FE // NQT              # 16 groups per quarter
JQ = 8 * FQ                 # 128: width of a quarter tile
GL = GPB * NCH * 128        # 4096: free width of one x block (fp16)
AL = mybir.AluOpType
AF = mybir.ActivationFunctionType
F32 = mybir.dt.float32


def _split_waits(nc, max_waits=1):
    """walrus in this env accepts at most one sync-wait per instruction;
    move extras onto preceding same-engine NoOps."""
    for fn in nc.m.functions:
        for blk in fn.blocks:
            new_list = []
            for inst in blk.instructions:
                si = inst.sync_info
                if si is not None and len(si.on_wait) > max_waits:
                    waits = list(si.on_wait)
                    keep, extra = waits[-max_waits:], waits[:-max_waits]
                    for k, w in enumerate(extra):
                        new_list.append(mybir.InstNoOp(
                            name=f"{inst.name}-ws{k}", engine=inst.engine,
                            ins=[], outs=[],
                            sync_info=mybir.SyncInfo(on_wait=[w], on_update=[])))
                    si.on_wait = keep
                    inst.sync_info = si
                new_list.append(inst)
            blk.instructions = new_list


def _build_nc():
    nc = bass.Bass("TRN2", target_bir_lowering=False)
    # x relayout (hi-only fp16): xa[p, m*512 + k*128 + ms] = x[128m + ms, 128k + p]
    xa_d = nc.dram_tensor("xa", [128, BC * NCH], MM_DT, kind="ExternalInput").ap()
    # W chunks: [k*8 + j] = whi chunk k; [32 + k*8 + j] = wlo chunk k
    wcat_d = nc.dram_tensor("wcat", [128, 64], MM_DT, kind="ExternalInput").ap()
    # cvf: [0:16]=cv (W_cls/b_cls), then bsT | RT | aT each [128, JQ] (j*16 + u)
    cvf_d = nc.dram_tensor("cvf", [128, 16 + 3 * JQ], F32, kind="ExternalInput").ap()
    # o[p, 32*qi + 16*c + uq] = out_c(sample 128*(16*qi + uq) + p)
    o_d = nc.dram_tensor("o", [128, 2 * FE], F32, kind="ExternalOutput").ap()

    with TileContext(nc) as tc:
        with tc.tile_pool(name="wp", bufs=1) as wpool, \
             tc.tile_pool(name="xp", bufs=NBLK) as xpool, \
             tc.tile_pool(name="pp", bufs=4, space="PSUM") as pspool, \
             tc.tile_pool(name="ep", bufs=1) as epool:
            # constants on the ACT ring (parallel with x on the sync ring)
            wsb = wpool.tile([128, 64], MM_DT)
            nc.scalar.dma_start(wsb[:], wcat_d[:])
            cvsb = wpool.tile([128, 16 + 3 * JQ], F32)
            nc.scalar.dma_start(cvsb[:], cvf_d[:])
            cv = cvsb[:, 0:16]
            bsT = cvsb[:, 16:16 + JQ]
            RT = cvsb[:, 16 + JQ:16 + 2 * JQ]
            aT = cvsb[:, 16 + 2 * JQ:16 + 3 * JQ]
            bs3 = bsT.rearrange("p (j u) -> p j u", j=8)

            # all x DMAs up-front, one sequential queue
            Las = []
            for g in range(NBLK):
                La = xpool.tile([128, GL], MM_DT, tag="La", name=f"La{g}")
                nc.sync.dma_start(La[:], xa_d[:, g * GL:(g + 1) * GL])
                Las.append(La)

            for qi in range(NQT):
                E = epool.tile([128, JQ], F32, name=f"E{qi}")
                e3 = E.rearrange("p (j u) -> p j u", j=8)
                for lb in range(2):
                    g = 2 * qi + lb
                    La = Las[g]
                    ps = pspool.tile([128, GPB * 8], F32, tag="ps", name=f"ps{g}")
                    for mm in range(GPB):
                        for k in range(NCH):
                            off = mm * (NCH * 128) + k * 128
                            out_sl = ps[:, 8 * mm:8 * mm + 8]
                            nc.tensor.matmul(out_sl, La[:, off:off + 128],
                                             wsb[:, 8 * k:8 * k + 8],
                                             start=(k == 0), stop=(k == NCH - 1))
                    # E[:, j*FQ + 8*lb + m] = ps[:, 8m + j] + bs (phase shift)
                    nc.vector.tensor_add(e3[:, :, GPB * lb:GPB * (lb + 1)],
                                         ps.rearrange("p (m j) -> p j m", j=8),
                                         bs3[:, :, GPB * lb:GPB * (lb + 1)])

                # ---- epilogue for this quarter ----
                k2 = epool.tile([128, JQ], F32, name=f"k2_{qi}")
                r_ = epool.tile([128, JQ], F32, name=f"r_{qi}")
                s_ = epool.tile([128, JQ], F32, name=f"s_{qi}")
                t1 = epool.tile([128, JQ], F32, name=f"t1_{qi}")
                d_ = epool.tile([128, JQ], F32, name=f"d_{qi}")
                nc.vector.tensor_scalar(k2[:], E[:], M2, M2, AL.add, AL.subtract)
                nc.vector.tensor_sub(r_[:], E[:], k2[:])       # E mod 2 -> [-1, 1]
                nc.scalar.activation(s_[:], r_[:], AF.Sin, scale=PI)
                nc.vector.tensor_mul(t1[:], s_[:], RT)         # R sin
                nc.vector.tensor_add(d_[:], t1[:], aT)         # d = a + R sin

                def dj(r, i):
                    j = 4 * r + i
                    return d_[:, j * FQ:(j + 1) * FQ]

                # products: z_k^r at Z[:, (2k + r)*FQ]
                S_ = epool.tile([128, 2 * FQ], F32, name=f"S_{qi}")
                Z_ = epool.tile([128, 8 * FQ], F32, name=f"Z_{qi}")

                def zs(k, r):
                    return Z_[:, (2 * k + r) * FQ:(2 * k + r + 1) * FQ]

                for r in range(2):
                    u_ = S_[:, r * FQ:(r + 1) * FQ]
                    nc.vector.tensor_mul(u_, dj(r, 1), dj(r, 2))        # d1 d2
                    nc.vector.tensor_mul(zs(1, r), dj(r, 0), dj(r, 1))  # z1
                    nc.vector.tensor_mul(zs(2, r), dj(r, 0), u_)        # z2
                    nc.vector.tensor_mul(zs(0, r), u_, dj(r, 3))        # z0
                    nc.vector.tensor_mul(zs(3, r), zs(2, r), dj(r, 3))  # z3
                Mn = epool.tile([128, 4 * FQ], F32, name=f"Mn{qi}")
                for k in range(4):
                    nc.vector.tensor_add(Mn[:, k * FQ:(k + 1) * FQ], zs(k, 0), zs(k, 1))

                # final linear: W' = 0.5*W_cls via cv columns
                Oq = epool.tile([128, 2 * FQ], F32, name=f"Oq{qi}")
                O1 = epool.tile([128, 2 * FQ], F32, name=f"O1_{qi}")

                def mk(k):
                    return Mn[:, k * FQ:(k + 1) * FQ]

                for c in range(2):
                    o1a = O1[:, c * FQ:(c + 1) * FQ]
                    o1b = S_[:, c * FQ:(c + 1) * FQ]    # reuse S_ as scratch
                    oq = Oq[:, c * FQ:(c + 1) * FQ]
                    nc.vector.tensor_scalar(o1a, mk(0), cv[:, 4 + 4 * c:5 + 4 * c],
                                            cv[:, 12 + c:13 + c], AL.mult, AL.add)
                    nc.vector.scalar_tensor_tensor(o1b, mk(1), cv[:, 5 + 4 * c:6 + 4 * c],
                                                   o1a, AL.mult, AL.add)
                    nc.vector.scalar_tensor_tensor(o1a, mk(2), cv[:, 6 + 4 * c:7 + 4 * c],
                                                   o1b, AL.mult, AL.add)
                    nc.vector.scalar_tensor_tensor(oq, mk(3), cv[:, 7 + 4 * c:8 + 4 * c],
                                                   o1a, AL.mult, AL.add)
                nc.scalar.dma_start(o_d[:, 2 * FQ * qi:2 * FQ * (qi + 1)], Oq[:])

    return nc


_NC_CACHE = {}


def _get_nc(split=True):
    key = ("nc", split)
    if key not in _NC_CACHE:
        nc = _build_nc()
        if split:
            _split_waits(nc)
        _NC_CACHE[key] = nc
    return _NC_CACHE[key]


def _qubit_abc(q_params):
    """Exact (a_i, b_i, c_i) with d_i(theta) = a + b sin(theta) + c cos(theta)."""
    out = np.zeros((NQ, 3), np.float64)
    for i in range(NQ):
        pa, pb, pc = [float(v) for v in q_params[3 * i:3 * i + 3]]

        def rx(t):
            return np.array([[np.cos(t / 2), -1j * np.sin(t / 2)],
                             [-1j * np.sin(t / 2), np.cos(t / 2)]])

        def ry(t):
            return np.array([[np.cos(t / 2), -np.sin(t / 2)],
                             [np.sin(t / 2), np.cos(t / 2)]])

        def rz(t):
            return np.array([[np.exp(-0.5j * t), 0], [0, np.exp(0.5j * t)]])

        H = np.array([[1, 1], [1, -1]]) / np.sqrt(2)
        U = rz(pc) @ ry(pb) @ rx(pa)

        def dfun(theta):
            v = U @ ry(theta) @ H @ np.array([1.0, 0.0])
            pr = np.abs(v) ** 2
            return pr[0] - pr[1]

        d0, dpi, dh = dfun(0.0), dfun(np.pi), dfun(np.pi / 2)
        a = (d0 + dpi) / 2
        c = (d0 - dpi) / 2
        b = dh - a
        out[i] = (a, b, c)
    return out


def _make_consts(b_ctq, q_params, W_cls, b_cls):
    abc = _qubit_abc(q_params)
    cvf = np.zeros((128, 16 + 3 * JQ), np.float32)
    wp = 0.5 * np.asarray(W_cls, np.float64)      # mean over r folded in
    for c in range(2):
        for k in range(4):
            cvf[:, 4 + 4 * c + k] = np.float32(wp[c, k])
        cvf[:, 12 + c] = np.float32(b_cls[c])
    for j in range(8):
        i = j % 4
        a, b, c_ = abc[i]
        R = np.hypot(b, c_)
        phi = np.arctan2(c_, b)
        cvf[:, 16 + j * FQ:16 + (j + 1) * FQ] = np.float32(b_ctq[j] + phi / np.pi)
        cvf[:, 16 + JQ + j * FQ:16 + JQ + (j + 1) * FQ] = np.float32(R)
        cvf[:, 16 + 2 * JQ + j * FQ:16 + 2 * JQ + (j + 1) * FQ] = np.float32(a)
    return cvf


def make_in_maps(x, W_ctq, b_ctq, q_params, W_cls, b_cls):
    f16 = np.float16
    wt = np.asarray(W_ctq, np.float32).T                        # [512, 8]
    whi = wt.astype(f16)
    wlo = (wt - whi.astype(np.float32)).astype(f16)
    wcat = np.zeros((128, 64), f16)
    for k in range(NCH):
        wcat[:, 8 * k:8 * (k + 1)] = whi[128 * k:128 * (k + 1), :]
        wcat[:, 32 + 8 * k:32 + 8 * (k + 1)] = wlo[128 * k:128 * (k + 1), :]
    cvf = _make_consts(np.asarray(b_ctq, np.float32),
                       np.asarray(q_params, np.float32),
                       np.asarray(W_cls, np.float32),
                       np.asarray(b_cls, np.float32))
    x = np.asarray(x, np.float32)
    in_maps = []
    for c in range(NCORES):
        xs = x[c * BC:(c + 1) * BC]                             # [8192, 512]
        # relayout: [p, m*512 + k*128 + ms] = xs[128 m + ms, 128 k + p]
        xa = np.ascontiguousarray(
            xs.reshape(NG, 128, NCH, 128).transpose(3, 0, 2, 1)
            .reshape(128, BC * NCH).astype(f16))
        in_maps.append({"xa": xa, "wcat": wcat, "cvf": cvf})
    return in_maps


def assemble_output(results):
    out = np.empty((B, 2), np.float32)
    for core in range(NCORES):
        o = results[core]["o"]                                   # [128, 2*FE]
        # o[p, 32*qi + 16*c + uq] = out_c(sample 128*(16*qi + uq) + p)
        out[core * BC:(core + 1) * BC] = (
            o.reshape(128, NQT, 2, FQ).transpose(1, 3, 0, 2).reshape(BC, 2))
    return out


def kernel(x, W_ctq, b_ctq, q_params, W_cls, b_cls):
    nc = _get_nc()
    in_maps = make_in_maps(x, W_ctq, b_ctq, q_params, W_cls, b_cls)
    res = bass_utils.run_bass_kernel_spmd(nc, in_maps, core_ids=list(range(NCORES)))
    return assemble_output(res.results)
